# revision 17
# baseline (speedup 1.0000x reference)
"""Trainium2 Bass kernel for the MHA-with-diagonal-softmax module.

Computation (per batch b):
    q = rope(x @ Wq.T), k = rope(x @ Wk.T), v = x @ Wv.T      (per head, DH=128)
    sumexp[s,h] = sum_k exp(q_h[s] . k_h[k] * DH^-0.5)
    diag[s,h]   = q_h[s] . k_h[s] * DH^-0.5
    w = exp(diag) / sumexp
    out = (w * v) @ Wo.T

Sharding: 8 cores = 2 (batch) x 4 (head groups of 4 heads).
Each core computes q/k/v for its 4 heads in transposed [head_dim, seq]
layout, the per-position softmax-diagonal weights, and a partial output
projection (its heads' rows of Wo), written as 2 head-pair partials that
the host sums.

On-chip dtype is fp16 (same PE throughput as bf16, 8x lower rounding
error - matters because exp() amplifies absolute score error), with fp32
PSUM accumulation everywhere.
"""

import numpy as np
from contextlib import ExitStack

# Problem constants (hardcoded per harness contract).
B, S, D, H, DH = 2, 2048, 2048, 16, 128
HPC = 4            # heads per core
NHL = HPC * DH     # 512 local head dims per core
KB = D // 128      # 16 contraction blocks
SB = S // 128      # 16 seq blocks of 128
SC = S // 512      # 4 seq/emb chunks of 512
NCORES = 8

_CACHE = {}


def _build_nc():
    import concourse.bass as bass
    import concourse.tile as tile
    from concourse import bacc, mybir
    from concourse.masks import make_identity

    F16 = mybir.dt.float16
    F32 = mybir.dt.float32
    AF = mybir.ActivationFunctionType
    ALU = mybir.AluOpType
    AX = mybir.AxisListType

    # Bacc (not raw Bass): its compile() splits multi-sem waits into
    # event-semaphore instructions - HW allows at most 1 wait per inst.
    nc = bacc.Bacc("TRN2", target_bir_lowering=False, debug=False)

    xT = nc.dram_tensor("xT", [D, S], F16, kind="ExternalInput").ap()
    wq = nc.dram_tensor("wq", [D, NHL], F16, kind="ExternalInput").ap()
    wk = nc.dram_tensor("wk", [D, NHL], F16, kind="ExternalInput").ap()
    wv = nc.dram_tensor("wv", [D, NHL], F16, kind="ExternalInput").ap()
    wo = nc.dram_tensor("wo", [NHL, D], F16, kind="ExternalInput").ap()
    ropeA = nc.dram_tensor("ropeA", [128, S], F16, kind="ExternalInput").ap()
    ropeB = nc.dram_tensor("ropeB", [128, S], F16, kind="ExternalInput").ap()
    y = nc.dram_tensor("y", [2, S, D], F16, kind="ExternalOutput").ap()

    xT_r = xT.rearrange("(a p) s -> a p s", p=128)
    wq_r = wq.rearrange("(a p) m -> a p m", p=128)
    wk_r = wk.rearrange("(a p) m -> a p m", p=128)
    wv_r = wv.rearrange("(a p) m -> a p m", p=128)
    wo_r = wo.rearrange("(h p) n -> h p n", p=128)

    with tile.TileContext(nc) as tc, ExitStack() as ctx:
        pool = ctx.enter_context(tc.tile_pool(name="sb", bufs=1))
        pp = ctx.enter_context(tc.tile_pool(name="ps", bufs=1, space="PSUM"))

        # ---- constants ----
        ra = pool.tile([128, S], F16, name="ra")
        rb = pool.tile([128, S], F16, name="rb")
        # SWDGE: a wide HWDGE DMA fans out over several HW queues, and a
        # DVE/ACT consumer then needs one sync-wait per queue, exceeding
        # the instruction's wait-slot budget at compile time.
        nc.gpsimd.dma_start(ra[:, :], ropeA[:, :])
        nc.gpsimd.dma_start(rb[:, :], ropeB[:, :])
        ident = pool.tile([128, 128], F32, name="ident")
        make_identity(nc, ident[:, :])
        onesf = pool.tile([128, 128], F32, name="onesf")
        nc.gpsimd.memset(onesf[:, :], 1.0)
        ones1 = pool.tile([128, 128], F16, name="ones1")
        nc.gpsimd.memset(ones1[:, :], 1.0)

        # ---- x resident in SBUF ----
        xsb = pool.tile([128, KB, S], F16, name="xsb")
        for kb in range(KB):
            nc.sync.dma_start(xsb[:, kb, :], xT_r[kb])

        # ---- persistent q/k/v head tiles ([head_dim, seq] layout) ----
        qh = [pool.tile([128, S], F16, name=f"qh{h}") for h in range(HPC)]
        kh = [pool.tile([128, S], F16, name=f"kh{h}") for h in range(HPC)]
        vh = [pool.tile([128, S], F16, name=f"vh{h}") for h in range(HPC)]

        # per-head row vectors live at partition 32*h (engine ops only
        # support start partitions that are multiples of 32)
        ds_diag = pool.tile([128, S], F32, name="ds_diag")
        ds_sum = pool.tile([128, S], F16, name="ds_sum")
        w4 = pool.tile([128, S], F16, name="w4")
        sumf = [pool.tile([128, SB], F32, name=f"sumf{h}") for h in range(HPC)]

        def load_w(src_r, nblk, tag="w"):
            t = pool.tile([128, nblk, 512 * (KB // nblk)], F16, name="wt",
                          tag=tag, bufs=2)
            for i in range(nblk):
                nc.sync.dma_start(t[:, i, :], src_r[i])
            return t

        def proj(wt, dests):
            # dests[mt] [128, S] <- (wt[:, :, mt] block).T @ x
            for mt in range(HPC):
                for sc in range(SC):
                    ps = pp.tile([128, 512], F32, name="mmps", tag="mm", bufs=2)
                    for kb in range(KB):
                        nc.tensor.matmul(
                            ps[:, :],
                            wt[:, kb, mt * 128:(mt + 1) * 128],
                            xsb[:, kb, sc * 512:(sc + 1) * 512],
                            start=(kb == 0), stop=(kb == KB - 1))
                    nc.scalar.activation(
                        dests[mt][:, sc * 512:(sc + 1) * 512], ps[:, :], AF.Copy)

        def rope(dst):
            # dst (in place): top = te*cos - to*sin ; bottom = te*sin + to*cos
            # ra = [cosT; cosT], rb = [-sinT; sinT]; swap = halves exchanged.
            for c in range(2):
                sl = slice(c * 1024, (c + 1) * 1024)
                # SWDGE (gpsimd) keeps this 1 queue -> 1 sem; a wide HWDGE
                # sbuf->sbuf DMA fans out over many queues and blows the
                # consumer's sync-wait slot budget.
                swp = pool.tile([128, 1024], F16, name="swp", tag="swp", bufs=1)
                nc.gpsimd.dma_start(swp[0:64, :], dst[64:128, sl])
                nc.gpsimd.dma_start(swp[64:128, :], dst[0:64, sl])
                u = pool.tile([128, 1024], F16, name="u", tag="sc", bufs=2)
                nc.vector.tensor_mul(u[:, :], dst[:, sl], ra[:, sl])
                v2 = pool.tile([128, 1024], F16, name="v2", tag="sc", bufs=2)
                nc.vector.tensor_mul(v2[:, :], swp[:, :], rb[:, sl])
                nc.vector.tensor_add(dst[:, sl], u[:, :], v2[:, :])

        def diag(h):
            # ds_diag[32h, s] = sum_m qh[h][m, s] * kh[h][m, s]  (fp32)
            hp = 32 * h
            for c in range(2):
                sl = slice(c * 1024, (c + 1) * 1024)
                pr = pool.tile([128, 1024], F32, name="pr", tag="pr", bufs=1)
                nc.vector.tensor_mul(pr[:, :], qh[h][:, sl], kh[h][:, sl])
                for cc in range(2):
                    dps = pp.tile([128, 512], F32, name="dps", tag="sm", bufs=2)
                    nc.tensor.matmul(dps[:, :], onesf[:, :],
                                     pr[:, cc * 512:(cc + 1) * 512],
                                     start=True, stop=True)
                    o = (2 * c + cc) * 512
                    nc.scalar.activation(ds_diag[hp:hp + 1, o:o + 512],
                                         dps[hp:hp + 1, :], AF.Copy)

        # ================= phase 1: projections + rope =================
        wkt = load_w(wk_r, KB)
        wqt = load_w(wq_r, KB)
        proj(wkt, kh)
        for h in range(HPC):
            rope(kh[h])
        wvt = load_w(wv_r, KB)
        proj(wqt, qh)
        for h in range(HPC):
            rope(qh[h])
            diag(h)
        wot = load_w(wo_r, HPC)
        proj(wvt, vh)

        # ====== phase 2: scores -> exp/rowsum; per-pair w, attn, oproj ======
        def head_scores(h):
            for sq in range(SB):
                sumc = pool.tile([128, SC], F32, name="sumc", tag="sumc", bufs=4)
                for ck in range(SC):
                    sps = pp.tile([128, 512], F32, name="sps", tag="sco", bufs=4)
                    nc.tensor.matmul(sps[:, :],
                                     qh[h][:, sq * 128:(sq + 1) * 128],
                                     kh[h][:, ck * 512:(ck + 1) * 512],
                                     start=True, stop=True)
                    ex = pool.tile([128, 512], F16, name="ex", tag="ex", bufs=2)
                    nc.scalar.activation(ex[:, :], sps[:, :], AF.Exp,
                                         accum_out=sumc[:, ck:ck + 1])
                nc.vector.tensor_reduce(sumf[h][:, sq:sq + 1], sumc[:, :],
                                        axis=AX.X, op=ALU.add)
            # recip -> transpose -> [1, S] row of ds_sum
            rs = pool.tile([128, SB], F32, name="rs", tag="rs", bufs=2)
            nc.vector.reciprocal(rs[:, :], sumf[h][:, :])
            tps = pp.tile([16, 128], F32, name="tps", tag="sm", bufs=2)
            nc.tensor.transpose(tps[:, :], rs[:, :], ident[:, :])
            st = pool.tile([16, 128], F16, name="st", tag="st", bufs=2)
            nc.vector.tensor_copy(st[:, :], tps[:, :])
            nc.gpsimd.dma_start(ds_sum[32 * h:32 * h + 1, :], st[:, :])

        def pair_tail(p):
            h0, h1 = 2 * p, 2 * p + 1
            expd = pool.tile([128, S], F16, name="expd", tag="expd", bufs=2)
            for h in (h0, h1):
                hp = 32 * h
                nc.scalar.activation(expd[hp:hp + 1, :], ds_diag[hp:hp + 1, :],
                                     AF.Exp)
                nc.vector.tensor_mul(w4[hp:hp + 1, :], expd[hp:hp + 1, :],
                                     ds_sum[hp:hp + 1, :])
                # broadcast w4 row to all 128 partitions via K=1 matmul
                wb = pool.tile([128, S], F16, name="wb", tag="wb", bufs=1)
                for ck in range(SC):
                    bps = pp.tile([128, 512], F32, name="bps", tag="mm", bufs=2)
                    nc.tensor.matmul(bps[:, :], ones1[hp:hp + 1, :],
                                     w4[hp:hp + 1, ck * 512:(ck + 1) * 512],
                                     start=True, stop=True,
                                     tile_position=(hp, 0))
                    nc.scalar.activation(wb[:, ck * 512:(ck + 1) * 512],
                                         bps[:, :], AF.Copy)
                # attn overwrites kh[h] (dead after scores+diag)
                nc.vector.tensor_mul(kh[h][:, :], wb[:, :], vh[h][:, :])
            # output projection for this head pair -> y[p]
            for sb in range(SB):
                for ncx in range(SC):
                    ps = pp.tile([128, 512], F32, name="ops", tag="mm", bufs=2)
                    for i, h in enumerate((h0, h1)):
                        nc.tensor.matmul(
                            ps[:, :], kh[h][:, sb * 128:(sb + 1) * 128],
                            wot[:, h, ncx * 512:(ncx + 1) * 512],
                            start=(i == 0), stop=(i == 1))
                    yt = pool.tile([128, 512], F16, name="yt", tag="yt", bufs=2)
                    if (sb + ncx) % 2 == 0:
                        nc.scalar.activation(yt[:, :], ps[:, :], AF.Copy)
                    else:
                        nc.vector.tensor_copy(yt[:, :], ps[:, :])
                    nc.sync.dma_start(
                        y[p, sb * 128:(sb + 1) * 128,
                          ncx * 512:(ncx + 1) * 512], yt[:, :])

        head_scores(0)
        head_scores(1)
        pair_tail(0)
        head_scores(2)
        head_scores(3)
        pair_tail(1)

    nc.compile()
    return nc


def _get_nc():
    if "nc" not in _CACHE:
        _CACHE["nc"] = _build_nc()
    return _CACHE["nc"]


_PERM = np.concatenate([np.arange(0, DH, 2), np.arange(1, DH, 2)])


def _host_inputs(x, rope_cos, rope_sin, Wq, Wk, Wv, Wo):
    """Build the 8 per-core input maps."""
    f16 = np.float16
    cosT = np.ascontiguousarray(np.asarray(rope_cos, np.float32)[0, :, 0, :].T)
    sinT = np.ascontiguousarray(np.asarray(rope_sin, np.float32)[0, :, 0, :].T)
    ra = np.concatenate([cosT, cosT], 0).astype(f16)
    rb = np.concatenate([-sinT, sinT], 0).astype(f16)

    Wq = np.asarray(Wq, np.float32)
    Wk = np.asarray(Wk, np.float32)
    Wv = np.asarray(Wv, np.float32)
    Wo = np.asarray(Wo, np.float32)
    x = np.asarray(x, np.float32)

    xTb = [np.ascontiguousarray(x[b].T).astype(f16) for b in range(B)]
    scale = DH ** -0.5

    in_maps = []
    for core in range(NCORES):
        b, g = divmod(core, HPC)
        hs = g * HPC
        rows = np.concatenate(
            [h * DH + _PERM for h in range(hs, hs + HPC)])      # deinterleave
        rows_v = np.arange(hs * DH, (hs + HPC) * DH)
        in_maps.append({
            "xT": xTb[b],
            "wq": np.ascontiguousarray((Wq[rows] * scale).T).astype(f16),
            "wk": np.ascontiguousarray(Wk[rows].T).astype(f16),
            "wv": np.ascontiguousarray(Wv[rows_v].T).astype(f16),
            "wo": np.ascontiguousarray(Wo[:, rows_v].T).astype(f16),
            "ropeA": ra,
            "ropeB": rb,
        })
    return in_maps


def kernel(x, rope_cos, rope_sin, Wq, Wk, Wv, Wo, _trace=False, _trace_cores=None):
    from concourse.bass_utils import run_bass_kernel_spmd

    nc = _get_nc()
    in_maps = _host_inputs(x, rope_cos, rope_sin, Wq, Wk, Wv, Wo)
    res = run_bass_kernel_spmd(nc, in_maps, list(range(NCORES)),
                               trace=_trace, trace_cores=_trace_cores)
    _CACHE["last_result"] = res

    out = np.zeros((B, S, D), np.float32)
    for core in range(NCORES):
        b = core // HPC
        out[b] += res.results[core]["y"].astype(np.float32).sum(axis=0)
    return out
